# revision 10
# baseline (speedup 1.0000x reference)
"""Trainium2 Bass kernel for DiscriminatorAugment (B=128, C=3, H=W=256).

Data-parallel across 8 NeuronCores: 16 samples per core.

Math (per sample, derived from the reference):
    x0   = flip(images) if (flip & apply) else images       (done on HOST while staging)
    out  = apply ? mask_outside_box * (A*x0 + D3*g0sum + E_c) : images
with
    A    = s*c*b                (host scalar, 1.0 for bypassed samples)
    D3   = (1-s)*c*b / 3        (0.0 for bypassed)
    g0sum= x0_R + x0_G + x0_B   (per pixel, device)
    E_c  = alpha*S_c + beta3*(S_0+S_1+S_2)     (device; S_c = spatial SUM of channel c)
    alpha= b*(1-c)*s / (H*W),  beta3 = b*(1-c)*(1-s) / (3*H*W)   (0.0 for bypassed)
    mask : zero inside the (top,left)+64x64 cutout box (box pushed out of
           range for bypassed samples)

On-core layout: partition p = sample*8 + rowgroup (rowgroup = 32 rows),
free dim per chunk = [channel:3][row-in-chunk:8][w:256].  Per-sample scalars
become per-partition operand vectors; channel sums via PE matmuls with a 0/1
indicator stationary accumulating into PSUM; broadcast back to partitions via
a second tiny matmul.  The program is identical for every core (SPMD); all
per-sample behavior is carried by input data.
"""

import os
import sys
from contextlib import ExitStack

import numpy as np

for _p in ("/opt/trn_rl_repo", os.path.expanduser("~/.axon_site/_ro/trn_rl_repo")):
    if os.path.isdir(_p) and _p not in sys.path:
        sys.path.append(_p)

import concourse.bass as bass
import concourse.bacc as bacc
import concourse.tile as tile
from concourse import mybir

# problem constants
B, C, H, W = 128, 3, 256, 256
PROB = 0.9
BRI = CON = SAT = 0.2
CH = CW = 64
NCORES = 8
SPC = B // NCORES          # 16 samples per core
RG = 8                     # row groups per sample -> SPC*RG = 128 partitions
RGR = H // RG              # 32 rows per row group
NT = 4                     # pixel chunks
TR = RGR // NT             # 8 rows per chunk per rowgroup
PX = TR * W                # 2048 px per channel per partition per chunk
PXC = RGR * W              # 8192 px per channel per partition total
NPX = H * W                # 65536

# cst column map
COL_A, COL_D3, COL_TOP, COL_TOP64, COL_LEFT, COL_LEFT64, COL_AL, COL_BE = range(8)
COL_S = 8            # [8, 24)   indicator S[p, j] = (p//8 == j)
COL_S2 = 24          # [24, 152) rows 0..15: S2[j, p] = (p//8 == j)
COL_ROW = 152        # [152, 216) rowidx[p, q] = (p % 8)*32 + q, q in [0,64)
COL_CIDX = 216       # [216, 472) colidx[p, w] = w
NCOL = 472

F32 = mybir.dt.float32
ALU = mybir.AluOpType
ACT = mybir.ActivationFunctionType

_CACHE: dict = {}


def _build_nc() -> bass.Bass:
    # Bacc (not plain Bass): its compile() pass converts multi-sem waits to
    # event semaphores; this container's walrus rejects >1 embedded sem wait.
    nc = bacc.Bacc("TRN2", target_bir_lowering=False)
    ximg = nc.declare_dram_parameter("ximg", [SPC, C, H, W], F32, isOutput=False)
    cst = nc.declare_dram_parameter("cst", [128, NCOL], F32, isOutput=False)
    yout = nc.declare_dram_parameter("yout", [SPC, C, H, W], F32, isOutput=True)

    # dram view for chunk t, channel c: [b, rg, r*w] (3 dims so DMA can balance)
    def dram_chunk(tensor, t, c):
        v = tensor[:].rearrange(
            "b c (rg t r) w -> b rg t c (r w)", rg=RG, t=NT, r=TR
        )
        return v[:, :, t, c]

    with ExitStack() as ctx:
        tc = ctx.enter_context(tile.TileContext(nc))
        cpool = ctx.enter_context(tc.tile_pool(name="cst", bufs=1))
        xpool = ctx.enter_context(tc.tile_pool(name="xf", bufs=1))
        gpool = ctx.enter_context(tc.tile_pool(name="g0", bufs=2))
        ypool = ctx.enter_context(tc.tile_pool(name="y", bufs=2))
        mpool = ctx.enter_context(tc.tile_pool(name="mask", bufs=2))
        spool = ctx.enter_context(tc.tile_pool(name="small", bufs=1))
        pspool = ctx.enter_context(tc.tile_pool(name="psum", bufs=1, space="PSUM"))

        cst_sb = cpool.tile([128, NCOL], F32)
        nc.sync.dma_start(cst_sb[:], cst[:])

        avec = cst_sb[:, COL_A : COL_A + 1]
        d3vec = cst_sb[:, COL_D3 : COL_D3 + 1]
        topv = cst_sb[:, COL_TOP : COL_TOP + 1]
        top64v = cst_sb[:, COL_TOP64 : COL_TOP64 + 1]
        leftv = cst_sb[:, COL_LEFT : COL_LEFT + 1]
        left64v = cst_sb[:, COL_LEFT64 : COL_LEFT64 + 1]
        al16 = cst_sb[0:SPC, COL_AL : COL_AL + 1]
        be16 = cst_sb[0:SPC, COL_BE : COL_BE + 1]
        s_fwd = cst_sb[:, COL_S : COL_S + SPC]                 # [128, 16]
        s_bc = cst_sb[0:SPC, COL_S2 : COL_S2 + 128]            # [16, 128]
        colidx = cst_sb[:, COL_CIDX : COL_CIDX + W]            # [128, 256]

        # ---- loads (one tile per (chunk, channel) so each consumer waits on
        # exactly one DMA) ----
        xf = [
            [xpool.tile([128, PX], F32, name=f"xf{t}c{c}", tag=f"xf{t}c{c}") for c in range(C)]
            for t in range(NT)
        ]
        for t in range(NT):
            for c in range(C):
                nc.sync.dma_start(xf[t][c][:], dram_chunk(ximg, t, c))

        # ---- per-(sample, channel) sums via PE ----
        # Warm-up matmul touching only cst_sb: fp32 matmuls self-load weights,
        # so one instruction can carry a single sync wait; this one absorbs
        # the cst DMA wait so the real matmuls only wait on their image DMA.
        warm_ps = pspool.tile([SPC, 1], F32, tag="warm")
        nc.tensor.matmul(warm_ps[:], s_fwd, cst_sb[:, 0:1], start=True, stop=True)

        pm = [pspool.tile([SPC, 512], F32, name=f"pm{c}", tag=f"pm{c}") for c in range(C)]
        nsl = PX // 512
        for t in range(NT):
            for c in range(C):
                for j in range(nsl):
                    nc.tensor.matmul(
                        pm[c][:],
                        s_fwd,
                        xf[t][c][:, j * 512 : (j + 1) * 512],
                        start=(t == 0 and j == 0),
                        stop=(t == NT - 1 and j == nsl - 1),
                    )

        # ---- E_c = alpha*S_c + beta3*(S_0+S_1+S_2), broadcast to partitions ----
        sums = spool.tile([SPC, 4], F32)
        for c in range(C):
            nc.vector.tensor_reduce(sums[:, c : c + 1], pm[c][:], mybir.AxisListType.X, ALU.add)
        nc.vector.tensor_reduce(sums[:, 3:4], sums[:, 0:3], mybir.AxisListType.X, ALU.add)
        tmp16 = spool.tile([SPC, 1], F32)
        nc.vector.tensor_scalar(tmp16[:], sums[:, 3:4], be16, None, ALU.mult)
        ecat = spool.tile([SPC, C], F32)
        for c in range(C):
            nc.vector.scalar_tensor_tensor(
                ecat[:, c : c + 1], sums[:, c : c + 1], al16, tmp16[:], ALU.mult, ALU.add
            )
        ebc_ps = pspool.tile([128, C], F32, tag="ebc")
        nc.tensor.matmul(ebc_ps[:], s_bc, ecat[:], start=True, stop=True)
        ebc = spool.tile([128, C], F32)
        nc.vector.tensor_copy(ebc[:], ebc_ps[:])

        # ---- column outside-mask (chunk independent) ----
        ctmp = spool.tile([128, W], F32)
        colout = spool.tile([128, W], F32)
        nc.vector.tensor_scalar(ctmp[:], colidx, leftv, None, ALU.is_lt)
        nc.vector.tensor_scalar(colout[:], colidx, left64v, None, ALU.is_ge)
        nc.vector.tensor_add(colout[:], colout[:], ctmp[:])

        # ---- main per-chunk pipeline ----
        for t in range(NT):
            g0 = gpool.tile([128, PX], F32, tag="g0")
            nc.vector.tensor_add(g0[:], xf[t][0][:], xf[t][1][:])
            nc.vector.tensor_add(g0[:], g0[:], xf[t][2][:])

            y = ypool.tile([128, C * PX], F32, tag="y")
            yv = y[:].rearrange("p (c k) -> p c k", c=C)
            for c in range(C):
                nc.scalar.activation(
                    yv[:, c], xf[t][c][:], ACT.Identity,
                    bias=ebc[:, c : c + 1], scale=avec,
                )

            g0b = g0[:].unsqueeze(1).broadcast_to([128, C, PX])
            nc.vector.scalar_tensor_tensor(yv, g0b, d3vec, yv, ALU.mult, ALU.add)

            # outside-box mask for this chunk's rows
            rtmp = spool.tile([128, TR], F32, tag="rtmp")
            rowout = spool.tile([128, TR], F32, tag="rowout")
            ridx = cst_sb[:, COL_ROW + t * TR : COL_ROW + (t + 1) * TR]
            nc.vector.tensor_scalar(rtmp[:], ridx, topv, None, ALU.is_lt)
            nc.vector.tensor_scalar(rowout[:], ridx, top64v, None, ALU.is_ge)
            nc.vector.tensor_add(rowout[:], rowout[:], rtmp[:])

            outside = mpool.tile([128, PX], F32, tag="outside")
            ov = outside[:].rearrange("p (r w) -> p r w", r=TR)
            nc.vector.tensor_tensor(
                ov,
                rowout[:].unsqueeze(2).broadcast_to([128, TR, W]),
                colout[:].unsqueeze(1).broadcast_to([128, TR, W]),
                ALU.max,
            )

            ob = outside[:].unsqueeze(1).broadcast_to([128, C, PX])
            nc.gpsimd.tensor_tensor(yv, yv, ob, ALU.mult)

            for c in range(C):
                nc.sync.dma_start(dram_chunk(yout, t, c), yv[:, c])

    nc.finalize()
    return nc


def _get_nc() -> bass.Bass:
    if "nc" not in _CACHE:
        _CACHE["nc"] = _build_nc()
    return _CACHE["nc"]


def make_in_maps(images, apply_u, flip_u, brightness_u, contrast_u, saturation_u,
                 top_idx, left_idx):
    """Host-side staging: pre-flip flagged samples, build per-core constant
    tensors. Returns list of 8 in_maps."""
    images = np.ascontiguousarray(np.asarray(images, np.float32))
    apply_u = np.asarray(apply_u, np.float32)
    flip_u = np.asarray(flip_u, np.float32)
    bu = np.asarray(brightness_u, np.float32)
    cu = np.asarray(contrast_u, np.float32)
    su = np.asarray(saturation_u, np.float32)
    top_idx = np.asarray(top_idx)
    left_idx = np.asarray(left_idx)

    ap = apply_u < PROB
    fl = (flip_u < 0.5) & ap
    b = 1.0 - BRI + 2.0 * BRI * bu
    c = 1.0 - CON + 2.0 * CON * cu
    s = 1.0 - SAT + 2.0 * SAT * su
    A = np.where(ap, s * c * b, 1.0).astype(np.float32)
    D3 = np.where(ap, (1.0 - s) * c * b / 3.0, 0.0).astype(np.float32)
    al = (np.where(ap, b * (1.0 - c) * s, 0.0) / NPX).astype(np.float32)
    be3 = (np.where(ap, b * (1.0 - c) * (1.0 - s), 0.0) / (3.0 * NPX)).astype(np.float32)
    top = np.where(ap, top_idx.astype(np.float64), 1e9)
    left = np.where(ap, left_idx.astype(np.float64), 1e9)
    top64 = np.where(ap, top_idx.astype(np.float64) + CH, 2e9)
    left64 = np.where(ap, left_idx.astype(np.float64) + CW, 2e9)

    xall = images.copy()
    xall[fl] = xall[fl][..., ::-1]

    p = np.arange(128)
    in_maps = []
    for k in range(NCORES):
        sl = slice(k * SPC, (k + 1) * SPC)
        cst = np.zeros((128, NCOL), np.float32)
        cst[:, COL_A] = np.repeat(A[sl], RG)
        cst[:, COL_D3] = np.repeat(D3[sl], RG)
        cst[:, COL_TOP] = np.repeat(top[sl], RG)
        cst[:, COL_TOP64] = np.repeat(top64[sl], RG)
        cst[:, COL_LEFT] = np.repeat(left[sl], RG)
        cst[:, COL_LEFT64] = np.repeat(left64[sl], RG)
        cst[0:SPC, COL_AL] = al[sl]
        cst[0:SPC, COL_BE] = be3[sl]
        cst[:, COL_S : COL_S + SPC] = (p[:, None] // RG == np.arange(SPC)[None, :])
        cst[0:SPC, COL_S2 : COL_S2 + 128] = (p[None, :] // RG == np.arange(SPC)[:, None])
        cst[:, COL_ROW : COL_ROW + RGR] = ((p % RG) * RGR)[:, None] + np.arange(RGR)[None, :]
        cst[:, COL_CIDX : COL_CIDX + W] = np.arange(W)[None, :]
        in_maps.append({"ximg": np.ascontiguousarray(xall[sl]), "cst": cst})
    return in_maps


def run(in_maps, trace=False):
    from concourse.bass_utils import run_bass_kernel_spmd

    nc = _get_nc()
    return run_bass_kernel_spmd(nc, in_maps, list(range(NCORES)), trace=trace)


def kernel(images, apply_u, flip_u, brightness_u, contrast_u, saturation_u,
           top_idx, left_idx):
    in_maps = make_in_maps(images, apply_u, flip_u, brightness_u, contrast_u,
                           saturation_u, top_idx, left_idx)
    res = run(in_maps, trace=False)
    return np.concatenate([r["yout"] for r in res.results], axis=0)


# revision 13
# speedup vs baseline: 1.0025x; 1.0025x over previous
"""Trainium2 Bass kernel for DiscriminatorAugment (B=128, C=3, H=W=256).

Data-parallel across 8 NeuronCores: 16 samples per core.

Math (per sample, derived from the reference):
    x0   = flip(images) if (flip & apply) else images       (done on HOST while staging)
    out  = apply ? mask_outside_box * (A*x0 + D3*g0sum + E_c) : images
with
    A    = s*c*b                (host scalar, 1.0 for bypassed samples)
    D3   = (1-s)*c*b / 3        (0.0 for bypassed)
    g0sum= x0_R + x0_G + x0_B   (per pixel, device)
    E_c  = alpha*S_c + beta3*(S_0+S_1+S_2)     (device; S_c = spatial SUM of channel c)
    alpha= b*(1-c)*s / (H*W),  beta3 = b*(1-c)*(1-s) / (3*H*W)   (0.0 for bypassed)
    mask : zero inside the (top,left)+64x64 cutout box (box pushed out of
           range for bypassed samples)

On-core layout: partition p = sample*8 + rowgroup (rowgroup = 32 rows),
free dim per chunk = [channel:3][row-in-chunk:8][w:256].  Per-sample scalars
become per-partition operand vectors; channel sums via PE matmuls with a 0/1
indicator stationary accumulating into PSUM; broadcast back to partitions via
a second tiny matmul.  The program is identical for every core (SPMD); all
per-sample behavior is carried by input data.
"""

import os
import sys
from contextlib import ExitStack

import numpy as np

for _p in ("/opt/trn_rl_repo", os.path.expanduser("~/.axon_site/_ro/trn_rl_repo")):
    if os.path.isdir(_p) and _p not in sys.path:
        sys.path.append(_p)

import concourse.bass as bass
import concourse.bacc as bacc
import concourse.tile as tile
from concourse import mybir

# problem constants
B, C, H, W = 128, 3, 256, 256
PROB = 0.9
BRI = CON = SAT = 0.2
CH = CW = 64
NCORES = 8
SPC = B // NCORES          # 16 samples per core
RG = 8                     # row groups per sample -> SPC*RG = 128 partitions
RGR = H // RG              # 32 rows per row group
NT = 4                     # pixel chunks
TR = RGR // NT             # 8 rows per chunk per rowgroup
PX = TR * W                # 2048 px per channel per partition per chunk
PXC = RGR * W              # 8192 px per channel per partition total
NPX = H * W                # 65536

# cst column map
COL_A, COL_D3, COL_TOP, COL_TOP64, COL_LEFT, COL_LEFT64, COL_AL, COL_BE = range(8)
COL_S = 8            # [8, 24)   indicator S[p, j] = (p//8 == j)
COL_S2 = 24          # [24, 152) rows 0..15: S2[j, p] = (p//8 == j)
COL_ROW = 152        # [152, 216) rowidx[p, q] = (p % 8)*32 + q, q in [0,64)
COL_CIDX = 216       # [216, 472) colidx[p, w] = w
NCOL = 472

F32 = mybir.dt.float32
ALU = mybir.AluOpType
ACT = mybir.ActivationFunctionType

_CACHE: dict = {}


def _build_nc() -> bass.Bass:
    # Bacc (not plain Bass): its compile() pass converts multi-sem waits to
    # event semaphores; this container's walrus rejects >1 embedded sem wait.
    nc = bacc.Bacc("TRN2", target_bir_lowering=False)
    ximg = nc.declare_dram_parameter("ximg", [SPC, C, H, W], F32, isOutput=False)
    cst = nc.declare_dram_parameter("cst", [128, NCOL], F32, isOutput=False)
    yout = nc.declare_dram_parameter("yout", [SPC, C, H, W], F32, isOutput=True)

    # dram view for chunk t, channel c: [b, rg, r*w] (3 dims so DMA can balance)
    def dram_chunk(tensor, t, c):
        v = tensor[:].rearrange(
            "b c (rg t r) w -> b rg t c (r w)", rg=RG, t=NT, r=TR
        )
        return v[:, :, t, c]

    with ExitStack() as ctx:
        tc = ctx.enter_context(tile.TileContext(nc))
        cpool = ctx.enter_context(tc.tile_pool(name="cst", bufs=1))
        xpool = ctx.enter_context(tc.tile_pool(name="xf", bufs=1))
        gpool = ctx.enter_context(tc.tile_pool(name="g0", bufs=2))
        ypool = ctx.enter_context(tc.tile_pool(name="y", bufs=2))
        mpool = ctx.enter_context(tc.tile_pool(name="mask", bufs=2))
        spool = ctx.enter_context(tc.tile_pool(name="small", bufs=1))
        pspool = ctx.enter_context(tc.tile_pool(name="psum", bufs=1, space="PSUM"))

        cst_sb = cpool.tile([128, NCOL], F32)
        nc.sync.dma_start(cst_sb[:], cst[:])

        avec = cst_sb[:, COL_A : COL_A + 1]
        d3vec = cst_sb[:, COL_D3 : COL_D3 + 1]
        topv = cst_sb[:, COL_TOP : COL_TOP + 1]
        top64v = cst_sb[:, COL_TOP64 : COL_TOP64 + 1]
        leftv = cst_sb[:, COL_LEFT : COL_LEFT + 1]
        left64v = cst_sb[:, COL_LEFT64 : COL_LEFT64 + 1]
        al16 = cst_sb[0:SPC, COL_AL : COL_AL + 1]
        be16 = cst_sb[0:SPC, COL_BE : COL_BE + 1]
        s_fwd = cst_sb[:, COL_S : COL_S + SPC]                 # [128, 16]
        s_bc = cst_sb[0:SPC, COL_S2 : COL_S2 + 128]            # [16, 128]
        colidx = cst_sb[:, COL_CIDX : COL_CIDX + W]            # [128, 256]

        # ---- loads (one tile per (chunk, channel) so each consumer waits on
        # exactly one DMA) ----
        xf = [
            [xpool.tile([128, PX], F32, name=f"xf{t}c{c}", tag=f"xf{t}c{c}") for c in range(C)]
            for t in range(NT)
        ]
        for t in range(NT):
            for c in range(C):
                nc.sync.dma_start(xf[t][c][:], dram_chunk(ximg, t, c))

        # ---- per-(sample, channel) sums via PE ----
        # Warm-up matmul touching only cst_sb: fp32 matmuls self-load weights,
        # so one instruction can carry a single sync wait; this one absorbs
        # the cst DMA wait so the real matmuls only wait on their image DMA.
        warm_ps = pspool.tile([SPC, 1], F32, tag="warm")
        nc.tensor.matmul(warm_ps[:], s_fwd, cst_sb[:, 0:1], start=True, stop=True)

        pm = [pspool.tile([SPC, 512], F32, name=f"pm{c}", tag=f"pm{c}") for c in range(C)]
        nsl = PX // 512
        for t in range(NT):
            for c in range(C):
                for j in range(nsl):
                    nc.tensor.matmul(
                        pm[c][:],
                        s_fwd,
                        xf[t][c][:, j * 512 : (j + 1) * 512],
                        start=(t == 0 and j == 0),
                        stop=(t == NT - 1 and j == nsl - 1),
                    )

        # ---- E_c = alpha*S_c + beta3*(S_0+S_1+S_2), broadcast to partitions ----
        sums = spool.tile([SPC, 4], F32)
        for c in range(C):
            nc.vector.tensor_reduce(sums[:, c : c + 1], pm[c][:], mybir.AxisListType.X, ALU.add)
        nc.vector.tensor_reduce(sums[:, 3:4], sums[:, 0:3], mybir.AxisListType.X, ALU.add)
        tmp16 = spool.tile([SPC, 1], F32)
        nc.vector.tensor_scalar(tmp16[:], sums[:, 3:4], be16, None, ALU.mult)
        ecat = spool.tile([SPC, C], F32)
        for c in range(C):
            nc.vector.scalar_tensor_tensor(
                ecat[:, c : c + 1], sums[:, c : c + 1], al16, tmp16[:], ALU.mult, ALU.add
            )
        ebc_ps = pspool.tile([128, C], F32, tag="ebc")
        nc.tensor.matmul(ebc_ps[:], s_bc, ecat[:], start=True, stop=True)
        ebc = spool.tile([128, C], F32)
        nc.vector.tensor_copy(ebc[:], ebc_ps[:])

        # ---- column outside-mask (chunk independent) ----
        ctmp = spool.tile([128, W], F32)
        colout = spool.tile([128, W], F32)
        nc.vector.tensor_scalar(ctmp[:], colidx, leftv, None, ALU.is_lt)
        nc.vector.tensor_scalar(colout[:], colidx, left64v, None, ALU.is_ge)
        nc.vector.tensor_add(colout[:], colout[:], ctmp[:])

        # full-size outside mask [128, RGR*W], built once
        rtmp = spool.tile([128, RGR], F32)
        rowout = spool.tile([128, RGR], F32)
        ridx = cst_sb[:, COL_ROW : COL_ROW + RGR]
        nc.vector.tensor_scalar(rtmp[:], ridx, topv, None, ALU.is_lt)
        nc.vector.tensor_scalar(rowout[:], ridx, top64v, None, ALU.is_ge)
        nc.vector.tensor_add(rowout[:], rowout[:], rtmp[:])
        outf = mpool.tile([128, PXC], F32, bufs=1)
        nc.vector.tensor_tensor(
            outf[:].rearrange("p (r w) -> p r w", r=RGR),
            rowout[:].unsqueeze(2).broadcast_to([128, RGR, W]),
            colout[:].unsqueeze(1).broadcast_to([128, RGR, W]),
            ALU.max,
        )

        # ---- main per-chunk pipeline ----
        SPLIT = 512  # px of each chunk masked on DVE; rest on GpSimd
        for t in range(NT):
            g0 = gpool.tile([128, PX], F32, tag="g0")
            nc.gpsimd.tensor_add(g0[:], xf[t][0][:], xf[t][1][:])
            nc.gpsimd.tensor_add(g0[:], g0[:], xf[t][2][:])

            y = ypool.tile([128, C * PX], F32, tag="y")
            yv = y[:].rearrange("p (c k) -> p c k", c=C)
            for c in range(C):
                nc.scalar.activation(
                    yv[:, c], xf[t][c][:], ACT.Identity,
                    bias=ebc[:, c : c + 1], scale=avec,
                )

            g0b = g0[:].unsqueeze(1).broadcast_to([128, C, PX])
            nc.vector.scalar_tensor_tensor(yv, g0b, d3vec, yv, ALU.mult, ALU.add)

            # apply outside-mask (chunk t = px [t*PX, (t+1)*PX) of outf),
            # split along px between DVE and GpSimd
            om = outf[:, t * PX : (t + 1) * PX]
            ob_lo = om[:, 0:SPLIT].unsqueeze(1).broadcast_to([128, C, SPLIT])
            ob_hi = om[:, SPLIT:PX].unsqueeze(1).broadcast_to([128, C, PX - SPLIT])
            nc.vector.tensor_tensor(yv[:, :, 0:SPLIT], yv[:, :, 0:SPLIT], ob_lo, ALU.mult)
            nc.gpsimd.tensor_tensor(yv[:, :, SPLIT:PX], yv[:, :, SPLIT:PX], ob_hi, ALU.mult)

            for c in range(C):
                nc.sync.dma_start(dram_chunk(yout, t, c), yv[:, c])

    nc.finalize()
    return nc


def _get_nc() -> bass.Bass:
    if "nc" not in _CACHE:
        _CACHE["nc"] = _build_nc()
    return _CACHE["nc"]


def make_in_maps(images, apply_u, flip_u, brightness_u, contrast_u, saturation_u,
                 top_idx, left_idx):
    """Host-side staging: pre-flip flagged samples, build per-core constant
    tensors. Returns list of 8 in_maps."""
    images = np.ascontiguousarray(np.asarray(images, np.float32))
    apply_u = np.asarray(apply_u, np.float32)
    flip_u = np.asarray(flip_u, np.float32)
    bu = np.asarray(brightness_u, np.float32)
    cu = np.asarray(contrast_u, np.float32)
    su = np.asarray(saturation_u, np.float32)
    top_idx = np.asarray(top_idx)
    left_idx = np.asarray(left_idx)

    ap = apply_u < PROB
    fl = (flip_u < 0.5) & ap
    b = 1.0 - BRI + 2.0 * BRI * bu
    c = 1.0 - CON + 2.0 * CON * cu
    s = 1.0 - SAT + 2.0 * SAT * su
    A = np.where(ap, s * c * b, 1.0).astype(np.float32)
    D3 = np.where(ap, (1.0 - s) * c * b / 3.0, 0.0).astype(np.float32)
    al = (np.where(ap, b * (1.0 - c) * s, 0.0) / NPX).astype(np.float32)
    be3 = (np.where(ap, b * (1.0 - c) * (1.0 - s), 0.0) / (3.0 * NPX)).astype(np.float32)
    top = np.where(ap, top_idx.astype(np.float64), 1e9)
    left = np.where(ap, left_idx.astype(np.float64), 1e9)
    top64 = np.where(ap, top_idx.astype(np.float64) + CH, 2e9)
    left64 = np.where(ap, left_idx.astype(np.float64) + CW, 2e9)

    xall = images.copy()
    xall[fl] = xall[fl][..., ::-1]

    p = np.arange(128)
    in_maps = []
    for k in range(NCORES):
        sl = slice(k * SPC, (k + 1) * SPC)
        cst = np.zeros((128, NCOL), np.float32)
        cst[:, COL_A] = np.repeat(A[sl], RG)
        cst[:, COL_D3] = np.repeat(D3[sl], RG)
        cst[:, COL_TOP] = np.repeat(top[sl], RG)
        cst[:, COL_TOP64] = np.repeat(top64[sl], RG)
        cst[:, COL_LEFT] = np.repeat(left[sl], RG)
        cst[:, COL_LEFT64] = np.repeat(left64[sl], RG)
        cst[0:SPC, COL_AL] = al[sl]
        cst[0:SPC, COL_BE] = be3[sl]
        cst[:, COL_S : COL_S + SPC] = (p[:, None] // RG == np.arange(SPC)[None, :])
        cst[0:SPC, COL_S2 : COL_S2 + 128] = (p[None, :] // RG == np.arange(SPC)[:, None])
        cst[:, COL_ROW : COL_ROW + RGR] = ((p % RG) * RGR)[:, None] + np.arange(RGR)[None, :]
        cst[:, COL_CIDX : COL_CIDX + W] = np.arange(W)[None, :]
        in_maps.append({"ximg": np.ascontiguousarray(xall[sl]), "cst": cst})
    return in_maps


def run(in_maps, trace=False):
    from concourse.bass_utils import run_bass_kernel_spmd

    nc = _get_nc()
    return run_bass_kernel_spmd(nc, in_maps, list(range(NCORES)), trace=trace)


def kernel(images, apply_u, flip_u, brightness_u, contrast_u, saturation_u,
           top_idx, left_idx):
    in_maps = make_in_maps(images, apply_u, flip_u, brightness_u, contrast_u,
                           saturation_u, top_idx, left_idx)
    res = run(in_maps, trace=False)
    return np.concatenate([r["yout"] for r in res.results], axis=0)


# revision 15
# speedup vs baseline: 1.2379x; 1.2347x over previous
"""Trainium2 Bass kernel for DiscriminatorAugment (B=128, C=3, H=W=256).

Data-parallel across 8 NeuronCores: 16 samples per core.

Math (per sample, derived from the reference):
    x0   = flip(images) if (flip & apply) else images     (done on HOST while staging)
    t_c  = x_c + rho*(x_0+x_1+x_2),  rho = (1-s)/(3s)     (E-independent!)
    E_c  = alpha' * sum_px(t_c)                            (exact identity)
    out  = apply ? mask_outside_box * (A*t_c + E_c) : images
with A = s*c*b, alpha' = b*(1-c)*s/(H*W); bypassed samples get A=1, rho=0,
alpha'=0 and an out-of-range cutout box, so out == images exactly.

On-core layout: partition p = sample*8 + rowgroup (rowgroup = 32 rows), free
dim per chunk = [channel:3][row-in-chunk:8][w:256].  Per-sample scalars are
per-partition operand vectors.  Phase 1 (no E needed): DMA chunk loads, g0
adds on GpSimd, fused scalar_tensor_tensor t=x+rho*g0 on DVE with accum_out
producing per-partition sums for free.  Tiny PE matmuls reduce the sums to
per-sample E and broadcast back.  Phase 2: ScalarE affine A*t+E (in-place),
cutout mask multiply (split DVE/GpSimd), store.  The program is identical on
every core (SPMD); all per-sample behavior is carried by input data.
"""

import os
import sys
from contextlib import ExitStack

import numpy as np

for _p in ("/opt/trn_rl_repo", os.path.expanduser("~/.axon_site/_ro/trn_rl_repo")):
    if os.path.isdir(_p) and _p not in sys.path:
        sys.path.append(_p)

import concourse.bass as bass
import concourse.bacc as bacc
import concourse.tile as tile
from concourse import mybir

# problem constants
B, C, H, W = 128, 3, 256, 256
PROB = 0.9
BRI = CON = SAT = 0.2
CH = CW = 64
NCORES = 8
SPC = B // NCORES          # 16 samples per core
RG = 8                     # row groups per sample -> SPC*RG = 128 partitions
RGR = H // RG              # 32 rows per row group
NT = 4                     # pixel chunks
TR = RGR // NT             # 8 rows per chunk per rowgroup
PX = TR * W                # 2048 px per channel per partition per chunk
PXC = RGR * W              # 8192 px per channel per partition total
NPX = H * W

# cst column map
COL_A, COL_RHO, COL_TOP, COL_TOP64, COL_LEFT, COL_LEFT64, COL_AL = range(7)
COL_S = 8            # [8, 24)    indicator S[p, j] = (p//8 == j)
COL_S2 = 24          # [24, 152)  rows 0..15: S2[j, p] = (p//8 == j)
COL_ROW = 152        # [152, 184) rowidx[p, q] = (p % 8)*32 + q, q in [0,32)
COL_CIDX = 184       # [184, 440) colidx[p, w] = w
NCOL = 440

F32 = mybir.dt.float32
BF16 = mybir.dt.bfloat16
ALU = mybir.AluOpType
ACT = mybir.ActivationFunctionType

# mask-apply px split per chunk: [0, MSPLIT) on DVE, rest on GpSimd
MSPLIT = 512

_CACHE: dict = {}


def _build_nc() -> bass.Bass:
    # Bacc (not plain Bass): its compile() pass converts multi-sem waits to
    # event semaphores; this container's walrus rejects >1 embedded sem wait.
    nc = bacc.Bacc("TRN2", target_bir_lowering=False)
    ximg = nc.declare_dram_parameter("ximg", [SPC, C, H, W], F32, isOutput=False)
    cst = nc.declare_dram_parameter("cst", [128, NCOL], F32, isOutput=False)
    yout = nc.declare_dram_parameter("yout", [SPC, C, H, W], F32, isOutput=True)

    # dram view for chunk t, channel c: [b, rg, r*w] (3 dims so DMA can balance)
    def dram_chunk(tensor, t, c):
        v = tensor[:].rearrange(
            "b c (rg t r) w -> b rg t c (r w)", rg=RG, t=NT, r=TR
        )
        return v[:, :, t, c]

    with ExitStack() as ctx:
        tc = ctx.enter_context(tile.TileContext(nc))
        cpool = ctx.enter_context(tc.tile_pool(name="cst", bufs=1))
        xpool = ctx.enter_context(tc.tile_pool(name="xf", bufs=2))
        gpool = ctx.enter_context(tc.tile_pool(name="g0", bufs=2))
        tpool = ctx.enter_context(tc.tile_pool(name="t", bufs=1))
        mpool = ctx.enter_context(tc.tile_pool(name="mask", bufs=1))
        spool = ctx.enter_context(tc.tile_pool(name="small", bufs=1))
        pspool = ctx.enter_context(tc.tile_pool(name="psum", bufs=1, space="PSUM"))

        cst_sb = cpool.tile([128, NCOL], F32)
        nc.sync.dma_start(cst_sb[:], cst[:])

        avec = cst_sb[:, COL_A : COL_A + 1]
        rhovec = cst_sb[:, COL_RHO : COL_RHO + 1]
        topv = cst_sb[:, COL_TOP : COL_TOP + 1]
        top64v = cst_sb[:, COL_TOP64 : COL_TOP64 + 1]
        leftv = cst_sb[:, COL_LEFT : COL_LEFT + 1]
        left64v = cst_sb[:, COL_LEFT64 : COL_LEFT64 + 1]
        al16 = cst_sb[0:SPC, COL_AL : COL_AL + 1]
        s_fwd = cst_sb[:, COL_S : COL_S + SPC]                 # [128, 16]
        s_bc = cst_sb[0:SPC, COL_S2 : COL_S2 + 128]            # [16, 128]
        ridx = cst_sb[:, COL_ROW : COL_ROW + RGR]              # [128, 32]
        colidx = cst_sb[:, COL_CIDX : COL_CIDX + W]            # [128, 256]

        # ---- outside-of-cutout mask [128, RGR*W] in bf16 (0/1 exact) ----
        ctmp = spool.tile([128, W], BF16)
        colout = spool.tile([128, W], BF16)
        nc.vector.tensor_scalar(ctmp[:], colidx, leftv, None, ALU.is_lt)
        nc.vector.tensor_scalar(colout[:], colidx, left64v, None, ALU.is_ge)
        nc.vector.tensor_add(colout[:], colout[:], ctmp[:])
        rtmp = spool.tile([128, RGR], BF16)
        rowout = spool.tile([128, RGR], BF16)
        nc.vector.tensor_scalar(rtmp[:], ridx, topv, None, ALU.is_lt)
        nc.vector.tensor_scalar(rowout[:], ridx, top64v, None, ALU.is_ge)
        nc.vector.tensor_add(rowout[:], rowout[:], rtmp[:])
        outf = mpool.tile([128, PXC], BF16)
        nc.vector.tensor_tensor(
            outf[:].rearrange("p (r w) -> p r w", r=RGR),
            rowout[:].unsqueeze(2).broadcast_to([128, RGR, W]),
            colout[:].unsqueeze(1).broadcast_to([128, RGR, W]),
            ALU.max,
        )

        # Warm-up matmul touching only cst_sb: fp32 matmuls self-load weights,
        # so one instruction carries a single sync wait; this one absorbs the
        # cst DMA wait so later matmuls only wait on their data producer.
        warm_ps = pspool.tile([SPC, 1], F32, tag="warm")
        nc.tensor.matmul(warm_ps[:], s_fwd, cst_sb[:, 0:1], start=True, stop=True)

        # ---- phase 1: load, g0 = R+G+B (GpSimd), t = x + rho*g0 (DVE, with
        # free per-partition sums via accum_out) ----
        xf = [
            [xpool.tile([128, PX], F32, name=f"xf{t}c{c}", tag=f"xfc{c}") for c in range(C)]
            for t in range(NT)
        ]
        tt = [tpool.tile([128, C * PX], F32, name=f"t{t}", tag=f"t{t}") for t in range(NT)]
        acc = spool.tile([128, C * NT], F32)
        for t in range(NT):
            for c in range(C):
                nc.sync.dma_start(xf[t][c][:], dram_chunk(ximg, t, c))
            g0 = gpool.tile([128, PX], F32, tag="g0")
            nc.gpsimd.tensor_add(g0[:], xf[t][0][:], xf[t][1][:])
            nc.gpsimd.tensor_add(g0[:], g0[:], xf[t][2][:])
            tv = tt[t][:].rearrange("p (c k) -> p c k", c=C)
            for c in range(C):
                nc.vector.scalar_tensor_tensor(
                    tv[:, c], g0[:], rhovec, xf[t][c][:],
                    ALU.mult, ALU.add,
                    accum_out=acc[:, t * C + c : t * C + c + 1],
                )

        # ---- E_c = alpha' * sum(t_c): reduce acc across rowgroups+chunks ----
        accr_ps = pspool.tile([SPC, C * NT], F32, tag="accr")
        nc.tensor.matmul(accr_ps[:], s_fwd, acc[:], start=True, stop=True)
        st16 = spool.tile([SPC, C], F32)
        # acc columns are (t, c); reduce over t via strided view (c, t)
        accr_v = accr_ps[:].rearrange("p (t c) -> p c t", t=NT)
        nc.vector.tensor_reduce(st16[:], accr_v, mybir.AxisListType.X, ALU.add)
        e16 = spool.tile([SPC, C], F32)
        nc.vector.tensor_scalar(e16[:], st16[:], al16, None, ALU.mult)
        ebc_ps = pspool.tile([128, C], F32, tag="ebc")
        nc.tensor.matmul(ebc_ps[:], s_bc, e16[:], start=True, stop=True)
        ebc = spool.tile([128, C], F32)
        nc.vector.tensor_copy(ebc[:], ebc_ps[:])

        # ---- phase 2: y = A*t + E (ScalarE, in-place), mask, store ----
        for t in range(NT):
            tv = tt[t][:].rearrange("p (c k) -> p c k", c=C)
            for c in range(C):
                nc.scalar.activation(
                    tv[:, c], tv[:, c], ACT.Identity,
                    bias=ebc[:, c : c + 1], scale=avec,
                )
            om = outf[:, t * PX : (t + 1) * PX]
            ob_lo = om[:, 0:MSPLIT].unsqueeze(1).broadcast_to([128, C, MSPLIT])
            ob_hi = om[:, MSPLIT:PX].unsqueeze(1).broadcast_to([128, C, PX - MSPLIT])
            nc.vector.tensor_tensor(
                tv[:, :, 0:MSPLIT], tv[:, :, 0:MSPLIT], ob_lo, ALU.mult
            )
            nc.gpsimd.tensor_tensor(
                tv[:, :, MSPLIT:PX], tv[:, :, MSPLIT:PX], ob_hi, ALU.mult
            )
            for c in range(C):
                nc.sync.dma_start(dram_chunk(yout, t, c), tv[:, c])

    nc.finalize()
    return nc


def _get_nc() -> bass.Bass:
    if "nc" not in _CACHE:
        _CACHE["nc"] = _build_nc()
    return _CACHE["nc"]


def make_in_maps(images, apply_u, flip_u, brightness_u, contrast_u, saturation_u,
                 top_idx, left_idx):
    """Host-side staging: pre-flip flagged samples, build per-core constant
    tensors. Returns list of 8 in_maps."""
    images = np.ascontiguousarray(np.asarray(images, np.float32))
    apply_u = np.asarray(apply_u, np.float32)
    flip_u = np.asarray(flip_u, np.float32)
    bu = np.asarray(brightness_u, np.float32)
    cu = np.asarray(contrast_u, np.float32)
    su = np.asarray(saturation_u, np.float32)
    top_idx = np.asarray(top_idx)
    left_idx = np.asarray(left_idx)

    ap = apply_u < PROB
    fl = (flip_u < 0.5) & ap
    b = 1.0 - BRI + 2.0 * BRI * bu
    c = 1.0 - CON + 2.0 * CON * cu
    s = 1.0 - SAT + 2.0 * SAT * su
    A = np.where(ap, s * c * b, 1.0).astype(np.float32)
    RHO = np.where(ap, (1.0 - s) / (3.0 * s), 0.0).astype(np.float32)
    al = (np.where(ap, b * (1.0 - c) * s, 0.0) / NPX).astype(np.float32)
    top = np.where(ap, top_idx.astype(np.float64), 1e9)
    left = np.where(ap, left_idx.astype(np.float64), 1e9)
    top64 = np.where(ap, top_idx.astype(np.float64) + CH, 2e9)
    left64 = np.where(ap, left_idx.astype(np.float64) + CW, 2e9)

    xall = images.copy()
    xall[fl] = xall[fl][..., ::-1]

    p = np.arange(128)
    in_maps = []
    for k in range(NCORES):
        sl = slice(k * SPC, (k + 1) * SPC)
        cst = np.zeros((128, NCOL), np.float32)
        cst[:, COL_A] = np.repeat(A[sl], RG)
        cst[:, COL_RHO] = np.repeat(RHO[sl], RG)
        cst[:, COL_TOP] = np.repeat(top[sl], RG)
        cst[:, COL_TOP64] = np.repeat(top64[sl], RG)
        cst[:, COL_LEFT] = np.repeat(left[sl], RG)
        cst[:, COL_LEFT64] = np.repeat(left64[sl], RG)
        cst[0:SPC, COL_AL] = al[sl]
        cst[:, COL_S : COL_S + SPC] = (p[:, None] // RG == np.arange(SPC)[None, :])
        cst[0:SPC, COL_S2 : COL_S2 + 128] = (p[None, :] // RG == np.arange(SPC)[:, None])
        cst[:, COL_ROW : COL_ROW + RGR] = ((p % RG) * RGR)[:, None] + np.arange(RGR)[None, :]
        cst[:, COL_CIDX : COL_CIDX + W] = np.arange(W)[None, :]
        in_maps.append({"ximg": np.ascontiguousarray(xall[sl]), "cst": cst})
    return in_maps


def run(in_maps, trace=False):
    from concourse.bass_utils import run_bass_kernel_spmd

    nc = _get_nc()
    return run_bass_kernel_spmd(nc, in_maps, list(range(NCORES)), trace=trace)


def kernel(images, apply_u, flip_u, brightness_u, contrast_u, saturation_u,
           top_idx, left_idx):
    in_maps = make_in_maps(images, apply_u, flip_u, brightness_u, contrast_u,
                           saturation_u, top_idx, left_idx)
    res = run(in_maps, trace=False)
    return np.concatenate([r["yout"] for r in res.results], axis=0)
